# revision 2
# baseline (speedup 1.0000x reference)
"""MoE (dense-activated, 32 experts) Trainium2 kernel.

Problem: out[b,t,u] = sum_e gate[b,t,e] * LeakyReLU((x @ We[e] + be[e]))[u]
         gate = x @ Wg + bg   (no softmax)
Shapes: x[32,512,128], Wg[128,32], bg[32], We[32,128,64], be[32,64] -> out[32,512,64]

Strategy: data-parallel over batch across 8 NeuronCores (4 batches = 2048
tokens per core), weights replicated, no collectives. Host pre-transposes
x so the contraction dim F=128 lands on SBUF partitions with contiguous
DMA; x and all weights ship as one concatenated [128, 4128] tensor
(weights-first chunked DMA so compute starts early, and so matmuls carry
at most one sync wait - Bacc splits the rest via event semaphores).

Per 128-token tile on device:
  PE : gate matmul [128,32] + 4 h-matmuls [128,512] in float32r
       (full 1 col/cycle rate with fp32 data; plain fp32 is 4x slower),
       stationary = xT tile slice, moving = [Wg | We_flat] in SBUF.
  ACT: gate copy PSUM->SBUF + LeakyReLU (Prelu w/ alpha; Lrelu is not in
       any HW act table) per 16-expert PSUM half group (2 banks, bufs=3).
  DVE: t1 = HL * gate in bf16 at 2x_1P mode: the gate comes out of the
       PE already pair-duplicated (host packs each Wg column twice, so the
       gate matmul emits [tok, e, 2]; one ACT copy converts to bf16) and
       hl/t1 are viewed as
       [p, e, u/2, 2] so every operand's INNERMOST AP dim is (2, step 1)
       packed - mode detection ignores the stride-0 broadcast middle dim.
       Then a 4-level bf16 add-tree over experts @2x and a final
       contiguous fp32-output add over the last expert pair.

Measured (8-core SPMD, R-slope method): ~47.4 us/core steady-state sweep,
rel err ~5e-3 vs the fp32 reference (bf16 hl+product rounding dominates).
GPSIMD offload of the multiply/adds measures 1.5-2x WORSE on real HW
despite the cost model favoring it - do not re-enable GPS_MULT/GPS_TREE.
"""

import os
import sys

import numpy as np

for _p in ("/opt/trn_rl_repo", os.path.expanduser("~/.axon_site/_ro/trn_rl_repo")):
    if os.path.isdir(_p) and _p not in sys.path:
        sys.path.insert(0, _p)

import concourse.bass as bass
import concourse.bacc as bacc
import concourse.tile as tile
from concourse import mybir
from concourse.bass_utils import run_bass_kernel_spmd

ALPHA = 0.01

B, T, F, U, E = 32, 512, 128, 64, 32
N_CORES = 8
TOK = (B // N_CORES) * T          # tokens per core = 2048
P = 128                           # tokens per tile
N_TILES = TOK // P                # 16
EU = E * U                        # 2048
W_COLS = E * U + 2 * E            # 2112 = [Wg-paired | We_flat]
E_HALF = E // 2                   # experts per PSUM half-group
HCOLS = E_HALF * U                # 1024

f32 = mybir.dt.float32
f32r = mybir.dt.float32r

# toggles for iteration
GPS_MULT = int(os.environ.get("GPS_MULT", "0"))  # 0/1/2 halves on GPSIMD
DT_T1 = (mybir.dt.bfloat16 if os.environ.get("T1_DT", "bf16") == "bf16"
         else mybir.dt.float32)
DT_HL = (mybir.dt.bfloat16 if os.environ.get("HL_DT", "bf16") == "bf16"
         else mybir.dt.float32)
DT_GS = (mybir.dt.bfloat16 if os.environ.get("GS_DT", "f32") == "bf16"
         else mybir.dt.float32)
TREE_LEVELS = int(os.environ.get("TREE", "4"))

_CACHED = {}


def _build_nc(reps=1):
    """Build the single-core SPMD Bass module."""
    from contextlib import ExitStack

    nc = bacc.Bacc("TRN2")
    # XW = [xT | Wg | We_flat] : one DMA -> one semaphore -> every matmul
    # carries at most one sync wait (HW limit on the fused LDW+MM struct).
    XW = nc.declare_dram_parameter("XW", [F, TOK + W_COLS], f32r, isOutput=False)
    O = nc.declare_dram_parameter("O", [TOK, U], f32, isOutput=True)

    with ExitStack() as ctx:
        tc = ctx.enter_context(tile.TileContext(nc))
        singles = ctx.enter_context(tc.tile_pool(name="singles", bufs=1))
        xp = ctx.enter_context(tc.tile_pool(name="xp", bufs=3))
        hlp = ctx.enter_context(tc.tile_pool(name="hlp", bufs=int(os.environ.get("HLP_BUFS", "6"))))
        t1p = ctx.enter_context(tc.tile_pool(name="t1p", bufs=int(os.environ.get("T1P_BUFS", "4"))))
        outp = ctx.enter_context(tc.tile_pool(name="outp", bufs=int(os.environ.get("OUTP_BUFS", "4"))))
        gsb = ctx.enter_context(tc.tile_pool(name="gsb", bufs=int(os.environ.get("GSB_BUFS", "4"))))
        ph = ctx.enter_context(tc.tile_pool(name="ph", bufs=3, space="PSUM"))
        pg = ctx.enter_context(tc.tile_pool(name="pg", bufs=2, space="PSUM"))

        GOFF0 = TOK           # gate weight column offset (paired, 64 wide)
        HOFF0 = TOK + 2 * E   # expert weight column offset
        # Preload x and all weights: [xT | Wg | We_flat]
        xw = singles.tile([F, TOK + W_COLS], f32r)
        ds = os.environ.get("DMA_SPLIT", "2")
        if ds == "2":
            # gate weights (tiny) + first x chunk first, so tile-0's gate
            # matmul starts ~3us earlier; expert weights + remaining x
            # stream in behind it
            nc.sync.dma_start(out=xw[:, GOFF0:HOFF0], in_=XW[:, GOFF0:HOFF0])
            nc.sync.dma_start(out=xw[:, 0:512], in_=XW[:, 0:512])
            nc.sync.dma_start(out=xw[:, HOFF0:HOFF0 + HCOLS],
                              in_=XW[:, HOFF0:HOFF0 + HCOLS])
            nc.sync.dma_start(out=xw[:, HOFF0 + HCOLS:],
                              in_=XW[:, HOFF0 + HCOLS:])
            for c in range(1, 4):
                s = c * 512
                nc.sync.dma_start(out=xw[:, s:s + 512], in_=XW[:, s:s + 512])
        elif ds == "1":
            nc.sync.dma_start(out=xw[:, TOK:], in_=XW[:, TOK:])
            for c in range(4):
                s = c * 512
                nc.sync.dma_start(out=xw[:, s:s + 512], in_=XW[:, s:s + 512])
        else:
            nc.sync.dma_start(out=xw[:], in_=XW[:])
        GOFF = GOFF0
        HOFF = HOFF0

        def emit_tile(i):
            xt_r = xw[:, i * P:(i + 1) * P]

            # gate matmul with pair-duplicated Wg: [tok, 2E] in PSUM,
            # so the paired-gate layout comes out of the PE for free
            g_ps = pg.tile([P, 2 * E], f32)
            nc.tensor.matmul(
                g_ps[:], lhsT=xt_r, rhs=xw[:, GOFF:GOFF + 2 * E],
                start=True, stop=True,
            )

            # full-tile T1 product buffer (SBUF)
            t1 = t1p.tile([P, EU], DT_T1)

            # gate copy to SBUF (gpsimd cannot read PSUM)
            pair = os.environ.get("PAIR", "1") == "1"
            if pair:
                # duplicate each gate value into adjacent bf16 pairs so the
                # multiply's gate operand has a packed (2, step1) innermost
                # dim -> DVE 2x_1P mode despite the broadcast middle dim
                g2 = gsb.tile([P, 2 * E], mybir.dt.bfloat16)
                nc.scalar.activation(
                    g2[:], g_ps[:], mybir.ActivationFunctionType.Copy)
            else:
                g_sb = gsb.tile([P, E], DT_GS)
                nc.scalar.activation(
                    g_sb[:], g_ps[:].rearrange(
                        "p (e two) -> p two e", two=2)[:, 0],
                    mybir.ActivationFunctionType.Copy)

            for h in range(2):  # two 16-expert half groups
                h_ps = ph.tile([P, HCOLS], f32)
                for j in range(2):  # two 512-col matmuls per half
                    c0 = HOFF + h * HCOLS + j * 512
                    nc.tensor.matmul(
                        h_ps[:, j * 512:(j + 1) * 512],
                        lhsT=xt_r,
                        rhs=xw[:, c0:c0 + 512],
                        start=True, stop=True,
                    )
                # LeakyReLU PSUM -> SBUF
                hl = hlp.tile([P, HCOLS], DT_HL)
                nc.scalar.activation(
                    hl[:], h_ps[:], mybir.ActivationFunctionType.Prelu,
                    alpha=ALPHA,
                )
                # t1[:, half] = hl * gate (gate broadcast over U)
                eng = nc.gpsimd if h >= 2 - GPS_MULT else nc.vector
                if pair:
                    hl4 = hl[:].rearrange(
                        "p (e u2 two) -> p e u2 two", e=E_HALF, two=2)
                    g24 = (g2[:].rearrange("p (e two) -> p e two", two=2)
                           [:, h * E_HALF:(h + 1) * E_HALF]
                           .unsqueeze(2)
                           .broadcast_to([P, E_HALF, U // 2, 2]))
                    t14 = (t1[:, h * HCOLS:(h + 1) * HCOLS]
                           .rearrange("p (e u2 two) -> p e u2 two",
                                      e=E_HALF, two=2))
                    eng.tensor_tensor(t14, hl4, g24, op=mybir.AluOpType.mult)
                else:
                    hl3 = hl[:].rearrange("p (e u) -> p e u", e=E_HALF)
                    gb = (g_sb[:, h * E_HALF:(h + 1) * E_HALF]
                          .unsqueeze(2).broadcast_to([P, E_HALF, U]))
                    t1h = (t1[:, h * HCOLS:(h + 1) * HCOLS]
                           .rearrange("p (e u) -> p e u", e=E_HALF))
                    eng.tensor_tensor(t1h, hl3, gb, op=mybir.AluOpType.mult)

            # bf16 add-tree halves the expert dim, then strided reduce
            cur = t1[:]
            width, ne = EU, E
            dma_lvls = os.environ.get("DMA_TREE", "")
            for lvl in range(TREE_LEVELS):
                width //= 2
                ne //= 2
                nxt = cur[:, 0:width]
                if str(lvl) in dma_lvls:
                    # idle DMA engines can fold tree levels via
                    # read-modify-write (accum_op=add)
                    nc.gpsimd.dma_start(
                        out=nxt, in_=cur[:, width:2 * width],
                        accum_op=mybir.AluOpType.add)
                else:
                    nc.vector.tensor_tensor(
                        nxt, cur[:, 0:width], cur[:, width:2 * width],
                        op=mybir.AluOpType.add)
                cur = nxt
            o_t = outp.tile([P, U], f32)
            if ne == 1:
                nc.vector.tensor_copy(o_t[:], cur)
            elif ne == 2 and os.environ.get("FINAL_ADD", "1") == "1":
                # final level as a contiguous fp32-out add: strided-input
                # reduce_sum is slower on HW than the cost model claims
                nc.vector.tensor_tensor(
                    o_t[:], cur[:, 0:U], cur[:, U:2 * U],
                    op=mybir.AluOpType.add)
            else:
                t1v = cur.rearrange("p (e u) -> p u e", e=ne)
                nc.vector.reduce_sum(o_t[:], t1v, axis=mybir.AxisListType.X)

            nc.sync.dma_start(out=O[i * P:(i + 1) * P, :], in_=o_t[:])

        if reps == 1:
            for i in range(N_TILES):
                emit_tile(i)
        else:
            # benchmark mode: repeat the whole sweep in a HW loop
            with tc.For_i(0, reps, 1):
                for i in range(N_TILES):
                    emit_tile(i)

    nc.finalize()
    return nc


def _numpy_fallback(x, Wg, bg, We, be):
    gate = np.einsum("btf,fe->bte", x, Wg) + bg
    h = np.einsum("btf,efu->btue", x, We) + be.T
    h = np.where(h >= 0, h, ALPHA * h)
    return np.einsum("btue,bte->btu", h, gate).astype(np.float32)


LAST_RESULTS = None


def prepare_in_maps(x, Wg, bg, We, be):
    # W = [Wg-paired | We_flat(e-major, u-minor)] : [128, 2112]
    W_all = np.concatenate(
        [np.repeat(Wg, 2, axis=1),
         We.transpose(1, 0, 2).reshape(F, E * U)], axis=1
    ).astype(np.float32)

    xs = x.reshape(N_CORES, TOK, F)
    return [
        {"XW": np.ascontiguousarray(
            np.concatenate([xs[c].T, W_all], axis=1))}
        for c in range(N_CORES)
    ]


def kernel(x, Wg, bg, We, be):
    x = np.asarray(x, dtype=np.float32)
    Wg = np.asarray(Wg, dtype=np.float32)
    bg = np.asarray(bg, dtype=np.float32)
    We = np.asarray(We, dtype=np.float32)
    be = np.asarray(be, dtype=np.float32)

    # device fast path assumes zero biases (true for this problem's inputs)
    if np.any(bg) or np.any(be):
        return _numpy_fallback(x, Wg, bg, We, be)

    if "nc" not in _CACHED:
        _CACHED["nc"] = _build_nc()
    nc = _CACHED["nc"]

    in_maps = prepare_in_maps(x, Wg, bg, We, be)

    global LAST_RESULTS
    res = run_bass_kernel_spmd(nc, in_maps, list(range(N_CORES)))
    LAST_RESULTS = res
    out = np.stack([res.results[c]["O"] for c in range(N_CORES)], axis=0)
    return out.reshape(B, T, U)

